# revision 24
# baseline (speedup 1.0000x reference)
"""DiceLoss kernel for Trainium2 (Bass/Tile), data-parallel over batch on 8 cores.

Problem: inputs [8, 21, 512, 512] f32 logits, targets [8, 512, 512] int,
smooth scalar. reference = mean_b dice_b with
  dice_b = 1 - (2*I_b + s) / (S_b + T_b + s)
where probs = softmax(inputs, axis=1),
  I_b = sum_pix probs[target]        (ignore_index=255 pixels excluded)
  S_b = sum probs * mask = sum mask  (softmax sums to 1 over classes)
  T_b = sum mask.

Device kernel (per core = one batch element), v4:
  z_pix = sum_c exp(x_c), split across engines:
    - A8Z planes arrive as fp8(e4m3); ACT computes e=exp(x) -> fp8; pairs of
      e-planes are summed into PSUM via fp8 DoubleRow identity matmuls
      (2 planes per instruction at 0.5 cyc/row).
    - D8 planes arrive as fp8; DVE computes e via the Schraudolph bit trick
      (one tensor_scalar: int16(x*128*log2e + magic) bitcast to bf16; 1x
      rate from the fp8 source).
    - D16 planes arrive as bf16; same Schraudolph TS at 4x rate.
    - DVE-produced e-planes are summed into the same PSUM via bf16 identity
      matmuls.
  g_pix = exp(x_target): the target-class logit xt is gathered on the host
    (pure data movement, like the reshape) and shipped as one fp8 plane;
    ACT exps it to bf16. Ignored pixels get xt=-88 -> g=0.
  r = 1/z via the int16 bit trick on DVE (K - bits(bf16(z))), then
  I = sum(g*r) via a fused STT with f32 accum; N = sum(t != 255).
  out = [I_half0, I_half1, N] per partition; the host sums the 128 partials
  and finishes dice (a handful of scalars).

Scheduling: DRAM inputs are partition-major ([P, planes, 2048]) so one
dma_start per plane-group moves 4-6KB contiguous per partition (fewer
descriptors + fewer DMA instructions). In the bench loop the epilogue is
software-pipelined: iteration k's epilogue (PSUM->bf16, recip, STT) runs at
the START of iteration k+1 against explicit A/B-buffered state, overlapping
k+1's DMA/ACT instead of serializing at the tail.

Error budget (validated bit-exact in numpy + HW probe): fp8 x quantization
and the Schraudolph/bit-recip approximations are ~2-5% per pixel but
pseudo-random across 262144 pixels and largely cancelling between numerator
and denominator: final rel err ~1e-4 (gate is 2e-2).
"""

import numpy as np
import ml_dtypes

B, C, H, W = 8, 21, 512, 512
HW = H * W           # 262144
P = 128              # SBUF partitions
FREE = HW // P       # 2048 free-dim elements per partition
N_CORES = 8
MM_N = 512           # matmul free-dim chunk (one PSUM bank of f32)
IGNORE = 255

A8Z = 12             # z-planes exp'd on ACT (fp8 wire, DoubleRow pairs)
D8 = 6               # z-planes on DVE via Schraudolph, fp8 wire (1x TS)
D16 = C - A8Z - D8   # z-planes on DVE via Schraudolph, bf16 wire (4x TS)
# GP_SCH (below) carves planes out of D16's count: set D16 manually when
# using it; plane budget must satisfy A8Z + D8 + GP_SCH + D16 == C.
ACT_GROUP = 4        # planes per ACT exp instruction (2 or 4)
COUNT_ON_GPSIMD = False  # N-count on GPSIMD instead of DVE
EPI_COPY_ACT = False     # epilogue PSUM->bf16 copy on ACT instead of DVE
DSTREAM_ON_ACT_RING = False  # d8/d16 DMAs on the scalar HWDGE ring
GP_SCH = 0           # d8 planes Schraudolph'd on GPSIMD instead of DVE

LOG2E = 1.4426950408889634
SCH_A = float(128 * LOG2E)        # Schraudolph scale
SCH_B = float(16256 - 7.0)        # Schraudolph magic (round-to-nearest HW)
RECIP_K = float(0x7EF2)           # reciprocal bit-trick magic

_STATE = {}


def _patch_tile_drain():
    """This neuronxcc build rejects >1 sync-wait per instruction ("Too many
    sync wait commands"). Split multi-wait instructions: hoist extra waits
    onto single-wait InstNoOps inserted just before, on the same engine."""
    import concourse.tile as tile
    from concourse.vector_clock import ScopedClock
    from concourse import mybir
    import bass_rust

    if getattr(tile.TileContext, "_ant_drain_patched", False):
        return

    _orig_lower = tile.TileContext._lower_ordered_insts

    def _lower_split(self, ordered):
        for insts in ordered.values():
            new = []
            for inst in insts:
                si = getattr(inst, "sync_info", None)
                eng = getattr(inst, "engine", None)
                if (
                    si is not None
                    and si.on_wait
                    and len(si.on_wait) > 1
                    and eng is not None
                    and eng != mybir.EngineType.Unassigned
                ):
                    waits = list(si.on_wait)
                    for w in waits[:-1]:
                        new.append(
                            mybir.InstNoOp(
                                name=self.nc.get_next_instruction_name(),
                                engine=eng,
                                bass_nofuse=True,
                                sync_info=bass_rust.SyncInfo(
                                    on_wait=[w], on_update=[]
                                ),
                            )
                        )
                    inst.sync_info = bass_rust.SyncInfo(
                        on_wait=[waits[-1]], on_update=list(si.on_update or [])
                    )
                new.append(inst)
            insts[:] = new
        return _orig_lower(self, ordered)

    tile.TileContext._lower_ordered_insts = _lower_split

    def _drain_and_barrier(self, tick_clock, wait_clock):
        drain_inst = self.nc.sync.drain()
        wait_clock.add_sem_waits(
            drain_inst.ins, ScopedClock({None: tick_clock.global_clock})
        )
        ins = drain_inst.ins
        si = ins.sync_info
        if si is not None and si.on_wait and len(si.on_wait) > 1:
            waits = list(si.on_wait)
            ins.sync_info = bass_rust.SyncInfo(
                on_wait=waits[:1], on_update=list(si.on_update or [])
            )
            for w in waits[1:]:
                extra = self.nc.sync.drain()
                extra.ins.sync_info = bass_rust.SyncInfo(on_wait=[w], on_update=[])
        self.nc.all_engine_barrier()
        assert self.sems is not None
        popped = self.nc._tile_sem_poison_stack.pop()
        assert popped is self._sem_poison
        self.nc.clear_and_free_semaphores(list(self.sems.allocated().values()))
        self.nc.all_engine_barrier()

    tile.TileContext._drain_and_barrier = _drain_and_barrier
    tile.TileContext._ant_drain_patched = True


def _build_nc(bench_reps=0, bench_inner=1, do_act=True, do_sch=True,
              do_pe=True, do_epi=True):
    import concourse.bass as bass
    import concourse.tile as tile
    from concourse import mybir

    _patch_tile_drain()

    bf16 = mybir.dt.bfloat16
    f32 = mybir.dt.float32
    i16 = mybir.dt.int16
    fp8 = mybir.dt.float8e4
    Alu = mybir.AluOpType
    Act = mybir.ActivationFunctionType

    assert A8Z + D8 + GP_SCH + D16 == C
    nc = bass.Bass()
    NP8 = A8Z + D8 + GP_SCH
    # Partition-major layout: one DMA per plane-group, contiguous in DRAM
    # per partition (bigger descriptors, fewer DMA instructions).
    # xb holds the D16 bf16 z-planes plus xt (the host-gathered target
    # logit) as its last plane.
    x8_d = nc.declare_dram_parameter("x8", [P, NP8, FREE], fp8, isOutput=False)
    xb_d = nc.declare_dram_parameter("xb", [P, D16 + 1, FREE], bf16,
                                     isOutput=False)
    t_d = nc.declare_dram_parameter("t", [P, FREE], bf16, isOutput=False)
    o_d = nc.declare_dram_parameter("out", [P, 3], f32, isOutput=True)

    identp_np = np.zeros((P, 2, P), dtype=ml_dtypes.float8_e4m3fn)
    for i in range(P):
        identp_np[i, 0, i] = 1.0
        identp_np[i, 1, i] = 1.0
    identp_d = nc.inline_tensor(identp_np, name="identp")
    ident_d = nc.inline_tensor(np.eye(P, dtype=ml_dtypes.bfloat16), name="ident")

    NSL = FREE // MM_N  # 4 psum-bank chunks per plane

    def _groups_f(n, base, gmax=3):
        out, c0 = [], 0
        while c0 < n:
            gsz = min(gmax, n - c0)
            out.append((base + c0, gsz))
            c0 += gsz
        return out

    # fp8 ACT groups (pairs of planes feed DoubleRow matmuls); DVE groups of
    # <=3 planes (fp8 and bf16 wires)
    assert A8Z % 2 == 0
    A_GROUPS = _groups_f(A8Z, 0, ACT_GROUP)

    D8_GROUPS = _groups_f(D8, A8Z)       # plane range within x8_d
    GP_GROUPS = _groups_f(GP_SCH, A8Z + D8)  # gpsimd planes within x8_d
    D16_GROUPS = _groups_f(D16, 0)       # plane range within xb_d

    # Interleave DVE/GPSIMD groups, then merge with ACT pairs. GPSIMD groups
    # lead: that engine is slow, so its planes need the longest runway.
    d_events = [("gp", i) for i in range(len(GP_GROUPS))]
    i8, i16g = 0, 0
    while i8 < len(D8_GROUPS) or i16g < len(D16_GROUPS):
        if i8 < len(D8_GROUPS):
            d_events.append(("d8", i8)); i8 += 1
        if i16g < len(D16_GROUPS):
            d_events.append(("d16", i16g)); i16g += 1
    EVENTS = []
    na, nd = len(A_GROUPS), len(d_events)
    ia, idx = 0, 0
    # two ACT pairs up front so the ACT engine is never starved by the
    # bigger DVE-stream transfers behind them in the DMA queue
    while ia < min(2, na):
        EVENTS.append(("a", ia)); ia += 1
    while ia < na or idx < nd:
        if idx < nd:
            EVENTS.append(d_events[idx]); idx += 1
        if ia < na:
            EVENTS.append(("a", ia)); ia += 1
    assert len(EVENTS) == na + nd

    with tile.TileContext(nc) as tc:
        with (
            tc.tile_pool(name="const", bufs=1) as constp,
            tc.tile_pool(name="state", bufs=1) as stp,
            tc.tile_pool(name="x8p", bufs=3 if ACT_GROUP <= 4 else 2) as x8p,
            tc.tile_pool(name="e8p", bufs=3 if ACT_GROUP <= 4 else 2) as e8p,
            tc.tile_pool(name="d8p", bufs=2) as d8p,
            tc.tile_pool(name="s8p", bufs=2) as s8p,
            tc.tile_pool(name="xbp", bufs=2) as xbp,
            tc.tile_pool(name="misc", bufs=2) as misc,
            tc.tile_pool(name="psum", bufs=1, space=bass.MemorySpace.PSUM) as psp,
        ):
            # Dummy [P,1] exp issued first: walrus inserts the ACT exp-table
            # load before it, so the load overlaps the first DMA.
            warm = misc.tile([P, 1], bf16)
            nc.vector.memset(warm[:], 0.0)
            warm2 = misc.tile([P, 1], bf16)
            nc.scalar.activation(warm2[:], warm[:], Act.Exp)

            # First x DMA goes first on the sync queue; constants ride the
            # GPSIMD HWDGE queue so they don't delay it.
            xt0 = x8p.tile([P, ACT_GROUP, FREE], fp8, tag="x8")
            g0 = min(ACT_GROUP, A8Z)
            nc.sync.dma_start(xt0[:, :g0, :], x8_d[:, 0:g0, :])
            identp = constp.tile([P, 2, P], fp8)
            nc.gpsimd.dma_start(identp[:], identp_d[:])
            ident = constp.tile([P, P], bf16)
            nc.gpsimd.dma_start(ident[:], ident_d[:])
            t_sb = misc.tile([P, FREE], bf16)
            nc.gpsimd.dma_start(t_sb[:], t_d[:])

            # Cross-iteration A/B state for the software-pipelined epilogue.
            zp_ab = [psp.tile([P, FREE], f32, tag=f"z{i}", name=f"zps{i}") for i in (0, 1)]
            g_ab = [stp.tile([P, FREE], bf16, tag=f"g{i}", name=f"gs{i}") for i in (0, 1)]
            outt_ab = [stp.tile([P, 3], f32, tag=f"outt{i}", name=f"outts{i}") for i in (0, 1)]

            def emit_epilogue(ph):
                # PSUM z -> bf16, bit-trick recip, I += g*r.
                zp, g, outt = zp_ab[ph], g_ab[ph], outt_ab[ph]
                zbf = misc.tile([P, FREE], bf16, tag="scratch")
                if EPI_COPY_ACT:
                    nc.scalar.activation(zbf[:], zp[:], Act.Copy)
                else:
                    nc.vector.tensor_copy(zbf[:], zp[:])
                rec = misc.tile([P, FREE], i16, tag="rec")
                nc.vector.tensor_scalar(
                    rec[:], zbf[:].bitcast(i16), RECIP_K, -1.0,
                    Alu.subtract, Alu.mult,
                )
                scr = misc.tile([P, FREE], bf16, tag="scratch")
                nc.vector.scalar_tensor_tensor(
                    scr[:], g[:], 0.0, rec[:].bitcast(bf16),
                    Alu.bypass, Alu.mult,
                    accum_out=outt[:, 0:1],
                )

            def emit_out(ph):
                # out-DMA emitted AFTER the next block's x-DMAs so it never
                # stalls the sync HWDGE ring on the epilogue's completion.
                nc.sync.dma_start(o_d[:], outt_ab[ph][:])

            def emit_producers(ph, xt0=None):
                zp, g, outt = zp_ab[ph], g_ab[ph], outt_ab[ph]

                # N = sum(t != 255), fused reduce on DVE
                scr2 = misc.tile([P, FREE], bf16, tag="scratch")
                if do_epi:
                    ceng = nc.gpsimd if COUNT_ON_GPSIMD else nc.vector
                    ceng.scalar_tensor_tensor(
                        scr2[:], t_sb[:], float(IGNORE), t_sb[:], Alu.not_equal,
                        Alu.bypass, accum_out=outt[:, 2:3],
                    )

                n_pe_groups = len(EVENTS)
                pe_seen = [0]

                def emit_pe(kind, tile_, gsz):
                    # accumulate gsz planes of tile_ into zp; per-512-chunk
                    # (one PSUM bank) accumulation chains, start on first,
                    # stop on last.
                    if not do_pe:
                        return
                    first = pe_seen[0] == 0
                    pe_seen[0] += 1
                    last = pe_seen[0] == n_pe_groups
                    if kind == "a":
                        npair = gsz // 2
                        for pr in range(npair):
                            pl = pr == npair - 1
                            for k in range(NSL):
                                nc.tensor.matmul(
                                    zp[:, bass.ts(k, MM_N)], identp[:],
                                    tile_[:, 2 * pr:2 * pr + 2,
                                          k * MM_N:(k + 1) * MM_N],
                                    start=first and pr == 0,
                                    stop=last and pl,
                                    perf_mode=mybir.MatmulPerfMode.DoubleRow,
                                )
                    else:
                        for h in range(gsz):
                            hl = h == gsz - 1
                            for k in range(NSL):
                                nc.tensor.matmul(
                                    zp[:, bass.ts(k, MM_N)], ident[:],
                                    tile_[:, h, k * MM_N:(k + 1) * MM_N],
                                    start=first and h == 0,
                                    stop=last and hl,
                                )

                pending = None  # deferred PE work: (kind, tile, gsz)
                for kind, gi in EVENTS:
                    if kind == "a":
                        c0, gsz = A_GROUPS[gi]
                        if gi == 0 and xt0 is not None:
                            xt = xt0
                        else:
                            xt = x8p.tile([P, ACT_GROUP, FREE], fp8, tag="x8")
                            nc.sync.dma_start(xt[:, :gsz, :],
                                              x8_d[:, c0:c0 + gsz, :])
                        e8 = e8p.tile([P, ACT_GROUP, FREE], fp8, tag="e8")
                        if do_act:
                            nc.scalar.activation(e8[:, :gsz, :], xt[:, :gsz, :],
                                                 Act.Exp)
                        prod = ("a", e8, gsz)
                    elif kind in ("d8", "gp"):
                        c0, gsz = (D8_GROUPS[gi] if kind == "d8"
                                   else GP_GROUPS[gi])
                        xd = d8p.tile([P, 3, FREE], fp8, tag=f"xd{kind}")
                        deng = nc.scalar if DSTREAM_ON_ACT_RING else nc.sync
                        deng.dma_start(xd[:, :gsz, :], x8_d[:, c0:c0 + gsz, :])
                        sch = s8p.tile([P, 3, FREE], i16, tag=f"sch8{kind}")
                        if do_sch:
                            seng = nc.vector if kind == "d8" else nc.gpsimd
                            seng.tensor_scalar(
                                sch[:, :gsz, :], xd[:, :gsz, :], SCH_A, SCH_B,
                                Alu.mult, Alu.add,
                            )
                        prod = ("d", sch[:].bitcast(bf16), gsz)
                    else:
                        c0, gsz = D16_GROUPS[gi]
                        xb = xbp.tile([P, 3, FREE], bf16, tag="xb")
                        deng = nc.scalar if DSTREAM_ON_ACT_RING else nc.sync
                        deng.dma_start(xb[:, :gsz, :], xb_d[:, c0:c0 + gsz, :])
                        if do_sch:
                            # in-place: bf16 and int16 are both 2B, so the
                            # Schraudolph bits overwrite the input tile
                            nc.vector.tensor_scalar(
                                xb[:, :gsz, :].bitcast(i16), xb[:, :gsz, :],
                                SCH_A, SCH_B, Alu.mult, Alu.add,
                            )
                        prod = ("d", xb[:], gsz)
                    if pending is not None:
                        emit_pe(*pending)
                    pending = prod
                # xt plane DMA + g via DVE Schraudolph straight into the
                # stable state tile (bits written through a bitcast view);
                # overlaps the PE tail / next epilogue.
                xtb = xbp.tile([P, 3, FREE], bf16, tag="xb")
                nc.sync.dma_start(xtb[:, 0, :], xb_d[:, D16, :])
                emit_pe(*pending)
                if do_sch:
                    nc.vector.tensor_scalar(
                        g[:].bitcast(i16), xtb[:, 0, :], SCH_A, SCH_B,
                        Alu.mult, Alu.add,
                    )

            if bench_reps:
                with tc.For_i(0, bench_reps, 1) as _i:
                    for j in range(bench_inner):
                        ph = j % 2
                        if do_epi and do_pe:
                            emit_epilogue(1 - ph)
                        emit_producers(ph)
                        if do_epi and do_pe:
                            emit_out(1 - ph)
            else:
                emit_producers(0, xt0=xt0)
                if do_epi and do_pe:
                    emit_epilogue(0)
                    emit_out(0)

    return nc


def _host_prep(inputs, targets):
    """Full inputs -> per-core input maps (layout/dtype transforms only)."""
    x = np.asarray(inputs, dtype=np.float32).reshape(B, C, HW)
    t = np.asarray(targets).reshape(B, HW)
    mask = t != IGNORE
    t_safe = np.where(mask, t, 0)
    n8 = A8Z + D8 + GP_SCH
    maps = []
    for b in range(B):
        xb_full = x[b]                                   # [C, HW]
        xt = np.take_along_axis(xb_full, t_safe[b][None, :], axis=0)[0]
        xt = np.where(mask[b], xt, np.float32(-88.0))
        x8 = np.ascontiguousarray(
            xb_full[:n8].reshape(n8, P, FREE).transpose(1, 0, 2)
        ).astype(ml_dtypes.float8_e4m3fn)
        xb16 = np.empty((P, D16 + 1, FREE), dtype=ml_dtypes.bfloat16)
        xb16[:, :D16, :] = (
            xb_full[n8:].reshape(D16, P, FREE).transpose(1, 0, 2)
            .astype(ml_dtypes.bfloat16)
        )
        xb16[:, D16, :] = xt.reshape(P, FREE).astype(ml_dtypes.bfloat16)
        tb = t[b].reshape(P, FREE).astype(ml_dtypes.bfloat16)
        maps.append({"x8": x8, "xb": xb16, "t": tb})
    return maps


def _build_runner():
    """Compile once; return fn(per_core_inputs) -> list of out arrays."""
    import jax
    from jax.sharding import Mesh, PartitionSpec
    from jax.experimental.shard_map import shard_map
    from concourse import bass2jax, mybir

    nc = _build_nc()
    bass2jax.install_neuronx_cc_hook()

    partition_name = nc.partition_id_tensor.name if nc.partition_id_tensor else None
    in_names = []
    out_names = []
    out_avals = []
    zero_outs = []
    for alloc in nc.m.functions[0].allocations:
        if not isinstance(alloc, mybir.MemoryLocationSet):
            continue
        name = alloc.memorylocations[0].name
        if alloc.kind == "ExternalInput":
            if name != partition_name:
                in_names.append(name)
        elif alloc.kind == "ExternalOutput":
            out_names.append(name)
            shape = tuple(alloc.tensor_shape)
            dtype = mybir.dt.np(alloc.dtype)
            out_avals.append(jax.core.ShapedArray(shape, dtype))
            zero_outs.append(np.zeros(shape, dtype))
    n_params = len(in_names)
    n_outs = len(out_avals)
    all_in_names = in_names + out_names
    if partition_name is not None:
        all_in_names = all_in_names + [partition_name]

    def _body(*args):
        operands = list(args)
        if partition_name is not None:
            operands.append(bass2jax.partition_id_tensor())
        outs = bass2jax._bass_exec_p.bind(
            *operands,
            out_avals=tuple(out_avals),
            in_names=tuple(all_in_names),
            out_names=tuple(out_names),
            lowering_input_output_aliases=(),
            sim_require_finite=False,
            sim_require_nnan=False,
            nc=nc,
        )
        return tuple(outs)

    devices = jax.devices()[:N_CORES]
    mesh = Mesh(np.asarray(devices), ("core",))
    in_specs = (PartitionSpec("core"),) * (n_params + n_outs)
    out_specs = (PartitionSpec("core"),) * n_outs
    donate = tuple(range(n_params, n_params + n_outs))
    sharded = jax.jit(
        shard_map(
            _body, mesh=mesh, in_specs=in_specs, out_specs=out_specs, check_rep=False
        ),
        donate_argnums=donate,
        keep_unused=True,
    )

    def run(per_core_in_maps):
        concat_in = [
            np.concatenate([m[name] for m in per_core_in_maps], axis=0)
            for name in in_names
        ]
        concat_zeros = [
            np.zeros((N_CORES * z.shape[0], *z.shape[1:]), z.dtype) for z in zero_outs
        ]
        out_arrs = sharded(*concat_in, *concat_zeros)
        return [
            np.asarray(out_arrs[0]).reshape(N_CORES, *out_avals[0].shape)[c]
            for c in range(N_CORES)
        ]

    return run


def _get_runner():
    if "runner" not in _STATE:
        _STATE["runner"] = _build_runner()
    return _STATE["runner"]


def kernel(inputs, targets, smooth):
    s = float(np.asarray(smooth))
    in_maps = _host_prep(inputs, targets)
    run = _get_runner()
    outs = run(in_maps)

    dices = []
    for b in range(B):
        ob = outs[b].astype(np.float64)
        I_b = ob[:, 0].sum()
        N_b = ob[:, 2].sum()
        dices.append(1.0 - (2.0 * I_b + s) / (2.0 * N_b + s))
    return np.float32(np.mean(dices))


# revision 25
# speedup vs baseline: 1.0228x; 1.0228x over previous
"""DiceLoss kernel for Trainium2 (Bass/Tile), data-parallel over batch on 8 cores.

Problem: inputs [8, 21, 512, 512] f32 logits, targets [8, 512, 512] int,
smooth scalar. reference = mean_b dice_b with
  dice_b = 1 - (2*I_b + s) / (S_b + T_b + s)
where probs = softmax(inputs, axis=1),
  I_b = sum_pix probs[target]        (ignore_index=255 pixels excluded)
  S_b = sum probs * mask = sum mask  (softmax sums to 1 over classes)
  T_b = sum mask.

Device kernel (per core = one batch element), v4:
  z_pix = sum_c exp(x_c), split across engines:
    - A8Z planes arrive as fp8(e4m3); ACT computes e=exp(x) -> fp8; pairs of
      e-planes are summed into PSUM via fp8 DoubleRow identity matmuls
      (2 planes per instruction at 0.5 cyc/row).
    - D8 planes arrive as fp8; DVE computes e via the Schraudolph bit trick
      (one tensor_scalar: int16(x*128*log2e + magic) bitcast to bf16; 1x
      rate from the fp8 source).
    - D16 planes arrive as bf16; same Schraudolph TS at 4x rate.
    - DVE-produced e-planes are summed into the same PSUM via bf16 identity
      matmuls.
  g_pix = exp(x_target): the target-class logit xt is gathered on the host
    (pure data movement, like the reshape) and shipped as one fp8 plane;
    ACT exps it to bf16. Ignored pixels get xt=-88 -> g=0.
  r = 1/z via the int16 bit trick on DVE (K - bits(bf16(z))), then
  I = sum(g*r) via a fused STT with f32 accum; N = sum(t != 255).
  out = [I_half0, I_half1, N] per partition; the host sums the 128 partials
  and finishes dice (a handful of scalars).

Scheduling: DRAM inputs are partition-major ([P, planes, 2048]) so one
dma_start per plane-group moves 4-6KB contiguous per partition (fewer
descriptors + fewer DMA instructions). In the bench loop the epilogue is
software-pipelined: iteration k's epilogue (PSUM->bf16, recip, STT) runs at
the START of iteration k+1 against explicit A/B-buffered state, overlapping
k+1's DMA/ACT instead of serializing at the tail.

Error budget (validated bit-exact in numpy + HW probe): fp8 x quantization
and the Schraudolph/bit-recip approximations are ~2-5% per pixel but
pseudo-random across 262144 pixels and largely cancelling between numerator
and denominator: final rel err ~1e-4 (gate is 2e-2).
"""

import numpy as np
import ml_dtypes

B, C, H, W = 8, 21, 512, 512
HW = H * W           # 262144
P = 128              # SBUF partitions
FREE = HW // P       # 2048 free-dim elements per partition
N_CORES = 8
MM_N = 512           # matmul free-dim chunk (one PSUM bank of f32)
IGNORE = 255

A8Z = 10             # z-planes exp'd on ACT (fp8 wire, DoubleRow pairs)
D8 = 5               # z-planes on DVE via Schraudolph, fp8 wire (1x TS)
D16 = 3              # z-planes on DVE via Schraudolph, bf16 wire (4x TS)
# GP_SCH (below) carves planes out of D16's count: set D16 manually when
# using it; plane budget must satisfy A8Z + D8 + GP_SCH + D16 == C.
ACT_GROUP = 4        # planes per ACT exp instruction (2 or 4)
COUNT_ON_GPSIMD = False  # N-count on GPSIMD instead of DVE
EPI_COPY_ACT = False     # epilogue PSUM->bf16 copy on ACT instead of DVE
DSTREAM_ON_ACT_RING = False  # d8/d16 DMAs on the scalar HWDGE ring
GP_SCH = 3           # fp8 planes Schraudolph'd on GPSIMD instead of DVE

LOG2E = 1.4426950408889634
SCH_A = float(128 * LOG2E)        # Schraudolph scale
SCH_B = float(16256 - 7.0)        # Schraudolph magic (round-to-nearest HW)
RECIP_K = float(0x7EF2)           # reciprocal bit-trick magic

_STATE = {}


def _patch_tile_drain():
    """This neuronxcc build rejects >1 sync-wait per instruction ("Too many
    sync wait commands"). Split multi-wait instructions: hoist extra waits
    onto single-wait InstNoOps inserted just before, on the same engine."""
    import concourse.tile as tile
    from concourse.vector_clock import ScopedClock
    from concourse import mybir
    import bass_rust

    if getattr(tile.TileContext, "_ant_drain_patched", False):
        return

    _orig_lower = tile.TileContext._lower_ordered_insts

    def _lower_split(self, ordered):
        for insts in ordered.values():
            new = []
            for inst in insts:
                si = getattr(inst, "sync_info", None)
                eng = getattr(inst, "engine", None)
                if (
                    si is not None
                    and si.on_wait
                    and len(si.on_wait) > 1
                    and eng is not None
                    and eng != mybir.EngineType.Unassigned
                ):
                    waits = list(si.on_wait)
                    for w in waits[:-1]:
                        new.append(
                            mybir.InstNoOp(
                                name=self.nc.get_next_instruction_name(),
                                engine=eng,
                                bass_nofuse=True,
                                sync_info=bass_rust.SyncInfo(
                                    on_wait=[w], on_update=[]
                                ),
                            )
                        )
                    inst.sync_info = bass_rust.SyncInfo(
                        on_wait=[waits[-1]], on_update=list(si.on_update or [])
                    )
                new.append(inst)
            insts[:] = new
        return _orig_lower(self, ordered)

    tile.TileContext._lower_ordered_insts = _lower_split

    def _drain_and_barrier(self, tick_clock, wait_clock):
        drain_inst = self.nc.sync.drain()
        wait_clock.add_sem_waits(
            drain_inst.ins, ScopedClock({None: tick_clock.global_clock})
        )
        ins = drain_inst.ins
        si = ins.sync_info
        if si is not None and si.on_wait and len(si.on_wait) > 1:
            waits = list(si.on_wait)
            ins.sync_info = bass_rust.SyncInfo(
                on_wait=waits[:1], on_update=list(si.on_update or [])
            )
            for w in waits[1:]:
                extra = self.nc.sync.drain()
                extra.ins.sync_info = bass_rust.SyncInfo(on_wait=[w], on_update=[])
        self.nc.all_engine_barrier()
        assert self.sems is not None
        popped = self.nc._tile_sem_poison_stack.pop()
        assert popped is self._sem_poison
        self.nc.clear_and_free_semaphores(list(self.sems.allocated().values()))
        self.nc.all_engine_barrier()

    tile.TileContext._drain_and_barrier = _drain_and_barrier
    tile.TileContext._ant_drain_patched = True


def _build_nc(bench_reps=0, bench_inner=1, do_act=True, do_sch=True,
              do_pe=True, do_epi=True):
    import concourse.bass as bass
    import concourse.tile as tile
    from concourse import mybir

    _patch_tile_drain()

    bf16 = mybir.dt.bfloat16
    f32 = mybir.dt.float32
    i16 = mybir.dt.int16
    fp8 = mybir.dt.float8e4
    Alu = mybir.AluOpType
    Act = mybir.ActivationFunctionType

    assert A8Z + D8 + GP_SCH + D16 == C
    nc = bass.Bass()
    NP8 = A8Z + D8 + GP_SCH
    # Partition-major layout: one DMA per plane-group, contiguous in DRAM
    # per partition (bigger descriptors, fewer DMA instructions).
    # xb holds the D16 bf16 z-planes plus xt (the host-gathered target
    # logit) as its last plane.
    x8_d = nc.declare_dram_parameter("x8", [P, NP8, FREE], fp8, isOutput=False)
    xb_d = nc.declare_dram_parameter("xb", [P, D16 + 1, FREE], bf16,
                                     isOutput=False)
    t_d = nc.declare_dram_parameter("t", [P, FREE], bf16, isOutput=False)
    o_d = nc.declare_dram_parameter("out", [P, 3], f32, isOutput=True)

    identp_np = np.zeros((P, 2, P), dtype=ml_dtypes.float8_e4m3fn)
    for i in range(P):
        identp_np[i, 0, i] = 1.0
        identp_np[i, 1, i] = 1.0
    identp_d = nc.inline_tensor(identp_np, name="identp")
    ident_d = nc.inline_tensor(np.eye(P, dtype=ml_dtypes.bfloat16), name="ident")

    NSL = FREE // MM_N  # 4 psum-bank chunks per plane

    def _groups_f(n, base, gmax=3):
        out, c0 = [], 0
        while c0 < n:
            gsz = min(gmax, n - c0)
            out.append((base + c0, gsz))
            c0 += gsz
        return out

    # fp8 ACT groups (pairs of planes feed DoubleRow matmuls); DVE groups of
    # <=3 planes (fp8 and bf16 wires)
    assert A8Z % 2 == 0
    A_GROUPS = _groups_f(A8Z, 0, ACT_GROUP)

    D8_GROUPS = _groups_f(D8, A8Z)       # plane range within x8_d
    GP_GROUPS = _groups_f(GP_SCH, A8Z + D8)  # gpsimd planes within x8_d
    D16_GROUPS = _groups_f(D16, 0)       # plane range within xb_d

    # Interleave DVE/GPSIMD groups, then merge with ACT pairs. GPSIMD groups
    # lead: that engine is slow, so its planes need the longest runway.
    d_events = [("gp", i) for i in range(len(GP_GROUPS))]
    i8, i16g = 0, 0
    while i8 < len(D8_GROUPS) or i16g < len(D16_GROUPS):
        if i8 < len(D8_GROUPS):
            d_events.append(("d8", i8)); i8 += 1
        if i16g < len(D16_GROUPS):
            d_events.append(("d16", i16g)); i16g += 1
    EVENTS = []
    na, nd = len(A_GROUPS), len(d_events)
    ia, idx = 0, 0
    # two ACT pairs up front so the ACT engine is never starved by the
    # bigger DVE-stream transfers behind them in the DMA queue
    while ia < min(2, na):
        EVENTS.append(("a", ia)); ia += 1
    while ia < na or idx < nd:
        if idx < nd:
            EVENTS.append(d_events[idx]); idx += 1
        if ia < na:
            EVENTS.append(("a", ia)); ia += 1
    assert len(EVENTS) == na + nd

    with tile.TileContext(nc) as tc:
        with (
            tc.tile_pool(name="const", bufs=1) as constp,
            tc.tile_pool(name="state", bufs=1) as stp,
            tc.tile_pool(name="x8p", bufs=3 if ACT_GROUP <= 4 else 2) as x8p,
            tc.tile_pool(name="e8p", bufs=3 if ACT_GROUP <= 4 else 2) as e8p,
            tc.tile_pool(name="d8p", bufs=2) as d8p,
            tc.tile_pool(name="s8p", bufs=2) as s8p,
            tc.tile_pool(name="xbp", bufs=2) as xbp,
            tc.tile_pool(name="misc", bufs=2) as misc,
            tc.tile_pool(name="psum", bufs=1, space=bass.MemorySpace.PSUM) as psp,
        ):
            # Dummy [P,1] exp issued first: walrus inserts the ACT exp-table
            # load before it, so the load overlaps the first DMA.
            warm = misc.tile([P, 1], bf16)
            nc.vector.memset(warm[:], 0.0)
            warm2 = misc.tile([P, 1], bf16)
            nc.scalar.activation(warm2[:], warm[:], Act.Exp)

            # First x DMA goes first on the sync queue; constants ride the
            # GPSIMD HWDGE queue so they don't delay it.
            xt0 = x8p.tile([P, ACT_GROUP, FREE], fp8, tag="x8")
            g0 = min(ACT_GROUP, A8Z)
            nc.sync.dma_start(xt0[:, :g0, :], x8_d[:, 0:g0, :])
            identp = constp.tile([P, 2, P], fp8)
            nc.gpsimd.dma_start(identp[:], identp_d[:])
            ident = constp.tile([P, P], bf16)
            nc.gpsimd.dma_start(ident[:], ident_d[:])
            t_sb = misc.tile([P, FREE], bf16)
            nc.gpsimd.dma_start(t_sb[:], t_d[:])

            # Cross-iteration A/B state for the software-pipelined epilogue.
            zp_ab = [psp.tile([P, FREE], f32, tag=f"z{i}", name=f"zps{i}") for i in (0, 1)]
            g_ab = [stp.tile([P, FREE], bf16, tag=f"g{i}", name=f"gs{i}") for i in (0, 1)]
            outt_ab = [stp.tile([P, 3], f32, tag=f"outt{i}", name=f"outts{i}") for i in (0, 1)]

            def emit_epilogue(ph):
                # PSUM z -> bf16, bit-trick recip, I += g*r.
                zp, g, outt = zp_ab[ph], g_ab[ph], outt_ab[ph]
                zbf = misc.tile([P, FREE], bf16, tag="scratch")
                if EPI_COPY_ACT:
                    nc.scalar.activation(zbf[:], zp[:], Act.Copy)
                else:
                    nc.vector.tensor_copy(zbf[:], zp[:])
                rec = misc.tile([P, FREE], i16, tag="rec")
                nc.vector.tensor_scalar(
                    rec[:], zbf[:].bitcast(i16), RECIP_K, -1.0,
                    Alu.subtract, Alu.mult,
                )
                scr = misc.tile([P, FREE], bf16, tag="scratch")
                nc.vector.scalar_tensor_tensor(
                    scr[:], g[:], 0.0, rec[:].bitcast(bf16),
                    Alu.bypass, Alu.mult,
                    accum_out=outt[:, 0:1],
                )

            def emit_out(ph):
                # out-DMA emitted AFTER the next block's x-DMAs so it never
                # stalls the sync HWDGE ring on the epilogue's completion.
                nc.sync.dma_start(o_d[:], outt_ab[ph][:])

            def emit_producers(ph, xt0=None):
                zp, g, outt = zp_ab[ph], g_ab[ph], outt_ab[ph]

                # N = sum(t != 255), fused reduce on DVE
                scr2 = misc.tile([P, FREE], bf16, tag="scratch")
                if do_epi:
                    ceng = nc.gpsimd if COUNT_ON_GPSIMD else nc.vector
                    ceng.scalar_tensor_tensor(
                        scr2[:], t_sb[:], float(IGNORE), t_sb[:], Alu.not_equal,
                        Alu.bypass, accum_out=outt[:, 2:3],
                    )

                n_pe_groups = len(EVENTS)
                pe_seen = [0]

                def emit_pe(kind, tile_, gsz):
                    # accumulate gsz planes of tile_ into zp; per-512-chunk
                    # (one PSUM bank) accumulation chains, start on first,
                    # stop on last.
                    if not do_pe:
                        return
                    first = pe_seen[0] == 0
                    pe_seen[0] += 1
                    last = pe_seen[0] == n_pe_groups
                    if kind == "a":
                        npair = gsz // 2
                        for pr in range(npair):
                            pl = pr == npair - 1
                            for k in range(NSL):
                                nc.tensor.matmul(
                                    zp[:, bass.ts(k, MM_N)], identp[:],
                                    tile_[:, 2 * pr:2 * pr + 2,
                                          k * MM_N:(k + 1) * MM_N],
                                    start=first and pr == 0,
                                    stop=last and pl,
                                    perf_mode=mybir.MatmulPerfMode.DoubleRow,
                                )
                    else:
                        for h in range(gsz):
                            hl = h == gsz - 1
                            for k in range(NSL):
                                nc.tensor.matmul(
                                    zp[:, bass.ts(k, MM_N)], ident[:],
                                    tile_[:, h, k * MM_N:(k + 1) * MM_N],
                                    start=first and h == 0,
                                    stop=last and hl,
                                )

                pending = None  # deferred PE work: (kind, tile, gsz)
                for kind, gi in EVENTS:
                    if kind == "a":
                        c0, gsz = A_GROUPS[gi]
                        if gi == 0 and xt0 is not None:
                            xt = xt0
                        else:
                            xt = x8p.tile([P, ACT_GROUP, FREE], fp8, tag="x8")
                            nc.sync.dma_start(xt[:, :gsz, :],
                                              x8_d[:, c0:c0 + gsz, :])
                        e8 = e8p.tile([P, ACT_GROUP, FREE], fp8, tag="e8")
                        if do_act:
                            nc.scalar.activation(e8[:, :gsz, :], xt[:, :gsz, :],
                                                 Act.Exp)
                        prod = ("a", e8, gsz)
                    elif kind in ("d8", "gp"):
                        c0, gsz = (D8_GROUPS[gi] if kind == "d8"
                                   else GP_GROUPS[gi])
                        xd = d8p.tile([P, 3, FREE], fp8, tag=f"xd{kind}")
                        deng = nc.scalar if DSTREAM_ON_ACT_RING else nc.sync
                        deng.dma_start(xd[:, :gsz, :], x8_d[:, c0:c0 + gsz, :])
                        sch = s8p.tile([P, 3, FREE], i16, tag=f"sch8{kind}")
                        if do_sch:
                            seng = nc.vector if kind == "d8" else nc.gpsimd
                            seng.tensor_scalar(
                                sch[:, :gsz, :], xd[:, :gsz, :], SCH_A, SCH_B,
                                Alu.mult, Alu.add,
                            )
                        prod = ("d", sch[:].bitcast(bf16), gsz)
                    else:
                        c0, gsz = D16_GROUPS[gi]
                        xb = xbp.tile([P, 3, FREE], bf16, tag="xb")
                        deng = nc.scalar if DSTREAM_ON_ACT_RING else nc.sync
                        deng.dma_start(xb[:, :gsz, :], xb_d[:, c0:c0 + gsz, :])
                        if do_sch:
                            # in-place: bf16 and int16 are both 2B, so the
                            # Schraudolph bits overwrite the input tile
                            nc.vector.tensor_scalar(
                                xb[:, :gsz, :].bitcast(i16), xb[:, :gsz, :],
                                SCH_A, SCH_B, Alu.mult, Alu.add,
                            )
                        prod = ("d", xb[:], gsz)
                    if pending is not None:
                        emit_pe(*pending)
                    pending = prod
                # xt plane DMA + g via DVE Schraudolph straight into the
                # stable state tile (bits written through a bitcast view);
                # overlaps the PE tail / next epilogue.
                xtb = xbp.tile([P, 3, FREE], bf16, tag="xb")
                nc.sync.dma_start(xtb[:, 0, :], xb_d[:, D16, :])
                emit_pe(*pending)
                if do_sch:
                    nc.vector.tensor_scalar(
                        g[:].bitcast(i16), xtb[:, 0, :], SCH_A, SCH_B,
                        Alu.mult, Alu.add,
                    )

            if bench_reps:
                with tc.For_i(0, bench_reps, 1) as _i:
                    for j in range(bench_inner):
                        ph = j % 2
                        if do_epi and do_pe:
                            emit_epilogue(1 - ph)
                        emit_producers(ph)
                        if do_epi and do_pe:
                            emit_out(1 - ph)
            else:
                emit_producers(0, xt0=xt0)
                if do_epi and do_pe:
                    emit_epilogue(0)
                    emit_out(0)

    return nc


def _host_prep(inputs, targets):
    """Full inputs -> per-core input maps (layout/dtype transforms only)."""
    x = np.asarray(inputs, dtype=np.float32).reshape(B, C, HW)
    t = np.asarray(targets).reshape(B, HW)
    mask = t != IGNORE
    t_safe = np.where(mask, t, 0)
    n8 = A8Z + D8 + GP_SCH
    maps = []
    for b in range(B):
        xb_full = x[b]                                   # [C, HW]
        xt = np.take_along_axis(xb_full, t_safe[b][None, :], axis=0)[0]
        xt = np.where(mask[b], xt, np.float32(-88.0))
        x8 = np.ascontiguousarray(
            xb_full[:n8].reshape(n8, P, FREE).transpose(1, 0, 2)
        ).astype(ml_dtypes.float8_e4m3fn)
        xb16 = np.empty((P, D16 + 1, FREE), dtype=ml_dtypes.bfloat16)
        xb16[:, :D16, :] = (
            xb_full[n8:].reshape(D16, P, FREE).transpose(1, 0, 2)
            .astype(ml_dtypes.bfloat16)
        )
        xb16[:, D16, :] = xt.reshape(P, FREE).astype(ml_dtypes.bfloat16)
        tb = t[b].reshape(P, FREE).astype(ml_dtypes.bfloat16)
        maps.append({"x8": x8, "xb": xb16, "t": tb})
    return maps


def _build_runner():
    """Compile once; return fn(per_core_inputs) -> list of out arrays."""
    import jax
    from jax.sharding import Mesh, PartitionSpec
    from jax.experimental.shard_map import shard_map
    from concourse import bass2jax, mybir

    nc = _build_nc()
    bass2jax.install_neuronx_cc_hook()

    partition_name = nc.partition_id_tensor.name if nc.partition_id_tensor else None
    in_names = []
    out_names = []
    out_avals = []
    zero_outs = []
    for alloc in nc.m.functions[0].allocations:
        if not isinstance(alloc, mybir.MemoryLocationSet):
            continue
        name = alloc.memorylocations[0].name
        if alloc.kind == "ExternalInput":
            if name != partition_name:
                in_names.append(name)
        elif alloc.kind == "ExternalOutput":
            out_names.append(name)
            shape = tuple(alloc.tensor_shape)
            dtype = mybir.dt.np(alloc.dtype)
            out_avals.append(jax.core.ShapedArray(shape, dtype))
            zero_outs.append(np.zeros(shape, dtype))
    n_params = len(in_names)
    n_outs = len(out_avals)
    all_in_names = in_names + out_names
    if partition_name is not None:
        all_in_names = all_in_names + [partition_name]

    def _body(*args):
        operands = list(args)
        if partition_name is not None:
            operands.append(bass2jax.partition_id_tensor())
        outs = bass2jax._bass_exec_p.bind(
            *operands,
            out_avals=tuple(out_avals),
            in_names=tuple(all_in_names),
            out_names=tuple(out_names),
            lowering_input_output_aliases=(),
            sim_require_finite=False,
            sim_require_nnan=False,
            nc=nc,
        )
        return tuple(outs)

    devices = jax.devices()[:N_CORES]
    mesh = Mesh(np.asarray(devices), ("core",))
    in_specs = (PartitionSpec("core"),) * (n_params + n_outs)
    out_specs = (PartitionSpec("core"),) * n_outs
    donate = tuple(range(n_params, n_params + n_outs))
    sharded = jax.jit(
        shard_map(
            _body, mesh=mesh, in_specs=in_specs, out_specs=out_specs, check_rep=False
        ),
        donate_argnums=donate,
        keep_unused=True,
    )

    def run(per_core_in_maps):
        concat_in = [
            np.concatenate([m[name] for m in per_core_in_maps], axis=0)
            for name in in_names
        ]
        concat_zeros = [
            np.zeros((N_CORES * z.shape[0], *z.shape[1:]), z.dtype) for z in zero_outs
        ]
        out_arrs = sharded(*concat_in, *concat_zeros)
        return [
            np.asarray(out_arrs[0]).reshape(N_CORES, *out_avals[0].shape)[c]
            for c in range(N_CORES)
        ]

    return run


def _get_runner():
    if "runner" not in _STATE:
        _STATE["runner"] = _build_runner()
    return _STATE["runner"]


def kernel(inputs, targets, smooth):
    s = float(np.asarray(smooth))
    in_maps = _host_prep(inputs, targets)
    run = _get_runner()
    outs = run(in_maps)

    dices = []
    for b in range(B):
        ob = outs[b].astype(np.float64)
        I_b = ob[:, 0].sum()
        N_b = ob[:, 2].sum()
        dices.append(1.0 - (2.0 * I_b + s) / (2.0 * N_b + s))
    return np.float32(np.mean(dices))
